# revision 1
# baseline (speedup 1.0000x reference)
"""Trainium2 Bass kernel for nn_DecoderBlock (dense transformer decoder block).

Sharding: 8 cores, zero collectives. Core c owns a contiguous block of
R = B*SQ/n_cores query rows of batch b = c // (n_cores/B). Each core
redundantly computes K/V projections for its whole batch (full kv length) and
runs the entire block on its own query rows. The host gathers row slices.

On-chip layout: activations are kept feature-major ("xT": [d, rows]) so every
linear layer is a plain matmul with the weight matrix (stored [in, out]) as the
stationary operand -- no on-chip transposes anywhere except the final output.
Attention scores are computed transposed ([k, q]); the softmax denominator is
obtained by appending a ones-column to V so the PV matmul also produces the
row sums. LayerNorm over the feature axis (= partitions) uses ones-vector
matmuls on the PE for sums / sums of squares.

All matmul operands are fp16 (e5m10) with fp32 PSUM accumulation: fp16 gets
the fast LDWEIGHTS path so back-to-back matmuls pipeline; fp32/fp32r matmuls
serialize LDWEIGHTS+MATMUL and run ~3x slower. All values here fit fp16 range
comfortably; the mask bias is -60000 (exp underflows to exactly 0).
"""

import threading
from contextlib import ExitStack
from dataclasses import dataclass

import numpy as np

import concourse.bass as bass
import concourse.tile as tile
from concourse import bacc, mybir
from concourse.bass_utils import run_bass_kernel_spmd
from concourse.masks import make_identity

F32 = mybir.dt.float32
F16 = mybir.dt.float16
AF = mybir.ActivationFunctionType
ALU = mybir.AluOpType

MASK_NEG = -60000.0


def _env_flag(name):
    import os
    return os.environ.get(name, "") not in ("", "0")


@dataclass(frozen=True)
class Cfg:
    B: int = 2
    SQ: int = 2048
    SKV: int = 2048
    D: int = 1024
    H: int = 16
    DFF: int = 4096
    n_cores: int = 8
    eps: float = 1e-5
    use_self_mask: bool = True
    use_cross_mask: bool = False

    @property
    def HD(self):
        return self.D // self.H

    @property
    def R(self):
        return self.B * self.SQ // self.n_cores

    @property
    def DC(self):
        return self.D // 128

    @property
    def KC(self):
        return self.SKV // 128

    @property
    def FC(self):
        return self.DFF // 128


def _col_view(ap_1d, p=128):
    return ap_1d.rearrange("(o p) -> p o", p=p)


def build_program(cfg: Cfg) -> bass.Bass:
    c = cfg
    use_bacc = not _env_flag("KB_NO_BACC")
    cls = bacc.Bacc if use_bacc else bass.Bass
    nc = cls("TRN2", target_bir_lowering=False, debug=False,
             num_devices=c.n_cores)
    scale = 1.0 / float(np.sqrt(c.HD))

    def din(name, shape, dtype=F32):
        return nc.dram_tensor(name, list(shape), dtype, kind="ExternalInput").ap()

    xT = din("xT", (c.D, c.SQ), F16)
    encT = din("encT", (c.D, c.SKV), F16)
    xTq = din("xTq", (c.D, c.R), F16)
    maskT = din("maskT", (c.SKV, c.R), F16) if c.use_self_mask else None
    maskcT = din("maskcT", (c.SKV, c.R), F16) if c.use_cross_mask else None
    W = {n: din(n, (c.D, c.D), F16) for n in
         ["wq_s", "wk_s", "wv_s", "wo_s", "wq_c", "wk_c", "wv_c", "wo_c"]}
    W["ff1_w"] = din("ff1_w", (c.D, c.DFF), F16)
    W["ff2_w"] = din("ff2_w", (c.DFF, c.D), F16)
    ones_in = din("ones_in", (128, c.H), F16)
    Bv = {n: din(n, (c.D,)) for n in
          ["bq_s", "bk_s", "bv_s", "bo_s", "bq_c", "bk_c", "bv_c", "bo_c",
           "ff2_b", "ln1_g", "ln1_b", "ln2_g", "ln2_b", "ln3_g", "ln3_b"]}
    Bv["ff1_b"] = din("ff1_b", (c.DFF,))
    out = nc.dram_tensor("out", [c.R, c.D], F32, kind="ExternalOutput").ap()

    def mm(ps, lhsT, rhs, start=True, stop=True):
        nc.tensor.matmul(ps, lhsT, rhs, start=start, stop=stop)

    with tile.TileContext(nc) as tc, ExitStack() as es:
        const = es.enter_context(tc.tile_pool(name="const", bufs=1))
        resid = es.enter_context(tc.tile_pool(name="resid", bufs=1))
        dram = es.enter_context(tc.tile_pool(name="dram", bufs=1, space="DRAM"))
        bncp = es.enter_context(tc.tile_pool(name="bnc", bufs=4, space="DRAM"))

        def bcast_rows(dst, src_row, n):
            """dst[p, :] = src_row[0, :] for all p, via a DRAM round-trip
            (a DMA with an SBUF source cannot partition-broadcast)."""
            bt = bncp.tile([1, n], F32, tag="bnc")
            nc.sync.dma_start(bt[:], src_row)
            nc.sync.dma_start(dst, bt[0:1, :].to_broadcast((dst.shape[0], n)))

        # ---- constants ----
        identity = const.tile([128, 128], F32, tag="identity")
        make_identity(nc, identity)
        ones_col = const.tile([128, 1], F16, tag="ones_col")
        nc.sync.dma_start(ones_col[:], ones_in[:, 0:1])
        ones_vp = const.tile([128, c.H], F16, tag="ones_vp")
        nc.sync.dma_start(ones_vp[:], ones_in[:, :])
        eps_t = const.tile([128, 1], F32, tag="eps")
        nc.vector.memset(eps_t, c.eps)

        def col(name):
            t = const.tile([128, Bv[name].shape[0] // 128], F32, tag=f"col_{name}")
            nc.sync.dma_start(t[:], _col_view(Bv[name]))
            return t

        cols = {n: col(n) for n in Bv}
        bv_bc = {}
        for n in ("bv_s", "bv_c"):
            t = const.tile([128, c.D], F32, tag=f"bcast_{n}")
            nc.sync.dma_start(t[:], Bv[n][None, :].to_broadcast((128, c.D)))
            bv_bc[n] = t

        # ---- DRAM scratch for K^T / V-interleaved staging (fp16) ----
        kt_s = dram.tile([c.DC, 128, c.SKV], F16, tag="kt_s")
        vp_s = dram.tile([c.KC, 128, c.H * (c.HD + 1)], F16, tag="vp_s")
        kt_c = dram.tile([c.DC, 128, c.SKV], F16, tag="kt_c")
        vp_c = dram.tile([c.KC, 128, c.H * (c.HD + 1)], F16, tag="vp_c")

        # ---- persistent activations ----
        h1T = resid.tile([128, c.DC, c.R], F16, tag="resid_a", name="h1T")
        h2T = resid.tile([128, c.DC, c.R], F16, tag="resid_b", name="h2T")
        h3T = resid.tile([128, c.DC, c.R], F32, tag="resid_c", name="h3T")

        dma_engs = [nc.sync, nc.gpsimd, nc.scalar]

        def load_weight(pool, w_ap, tag="w4m"):
            dn, dm = w_ap.shape
            ko = dn // 128
            t = pool.tile([128, ko, dm], F16, tag=tag)
            v = w_ap.rearrange("(o p) n -> p o n", p=128)
            step = max(1, -(-ko // 3))
            for idx, o in enumerate(range(0, ko, step)):
                e = min(o + step, ko)
                dma_engs[idx % 3].dma_start(t[:, o:e, :], v[:, o:e, :])
            return t

        def layernorm(pools, rT, outT, g_col, b_col):
            """outT = LN(rT)*g + b over the feature (partition) axis.

            All elementwise work happens on broadcast [128, n] tiles so the
            DVE/ACT lanes stay full (1-partition ops run on a single lane).
            """
            smallp, stage, bcp = pools["small"], pools["stage"], pools["bcast"]
            dc, n = rT.shape[1], rT.shape[2]
            dim = dc * 128
            with tc.tile_pool(name="ln_ps", bufs=1, space="PSUM") as pp:
                ps_sum = pp.tile([1, n], F32, tag="ln_sum")
                ps_sum2 = pp.tile([1, n], F32, tag="ln_sum2")
                for mo in range(dc):
                    mm(ps_sum, ones_col, rT[:, mo, :],
                       start=(mo == 0), stop=(mo == dc - 1))
                for mo in range(dc):
                    r2 = stage.tile([128, n], F16, tag="ln_r2")
                    nc.scalar.activation(out=r2, in_=rT[:, mo, :], func=AF.Square)
                    mm(ps_sum2, ones_col, r2, start=(mo == 0), stop=(mo == dc - 1))
                srow = smallp.tile([1, n], F32, tag="ln_srow")
                nc.vector.tensor_copy(out=srow, in_=ps_sum)
                s2row = smallp.tile([1, n], F32, tag="ln_s2row")
                nc.vector.tensor_copy(out=s2row, in_=ps_sum2)
            mean = bcp.tile([128, n], F32, tag="ln_mean")
            bcast_rows(mean[:], srow[0:1, :], n)
            var = bcp.tile([128, n], F32, tag="ln_var")
            bcast_rows(var[:], s2row[0:1, :], n)
            nc.vector.tensor_scalar_mul(mean, mean, 1.0 / dim)
            nc.vector.tensor_scalar_mul(var, var, 1.0 / dim)
            msq = bcp.tile([128, n], F32, tag="ln_msq")
            nc.vector.tensor_mul(out=msq, in0=mean, in1=mean)
            nc.vector.tensor_sub(out=var, in0=var, in1=msq)
            nc.scalar.activation(out=var, in_=var, func=AF.Sqrt,
                                 bias=eps_t[:, 0:1], scale=1.0)
            inv = bcp.tile([128, n], F32, tag="ln_inv")
            nc.vector.reciprocal(out=inv, in_=var)
            minv = bcp.tile([128, n], F32, tag="ln_minv")
            nc.vector.tensor_mul(out=minv, in0=mean, in1=inv)
            for mo in range(dc):
                t = stage.tile([128, n], F32, tag="ln_t")
                nc.vector.tensor_mul(out=t, in0=rT[:, mo, :], in1=inv)
                nc.vector.tensor_sub(out=t, in0=t, in1=minv)
                nc.vector.tensor_scalar(
                    out=outT[:, mo, :], in0=t,
                    scalar1=g_col[:, mo:mo + 1], scalar2=b_col[:, mo:mo + 1],
                    op0=ALU.mult, op1=ALU.add)

        def kv_proj(srcT_dram, wk_sb, wv_sb, bk_col, bvb, kt_dram, vp_dram):
            """Single pass over the kv sequence: K^T (feature-major fp16) and
            V (row-major fp16, ones-interleaved) staged to DRAM with one
            batched DMA per chunk each."""
            scw = min(512, c.SKV)
            nsc = c.SKV // scw
            voc = min(512, c.D)
            nvo = c.D // voc
            hper = voc // c.HD
            nkk = scw // 128
            with tc.tile_pool(name="kvx", bufs=2) as xpool, \
                 tc.tile_pool(name="kvstage", bufs=2) as stage, \
                 tc.tile_pool(name="kvps", bufs=4, space="PSUM") as pp:
                nc.scalar.dma_start(
                    vp_dram.rearrange("k p (h e) -> k p h e", e=c.HD + 1)
                    [:, :, :, c.HD:],
                    ones_in[None, :, :, None].to_broadcast((c.KC, 128, c.H, 1)))
                for sc in range(nsc):
                    xsb = xpool.tile([128, c.DC, scw], F16, tag="kv_x")
                    (nc.scalar if sc % 2 == 0 else nc.sync).dma_start(
                        xsb[:],
                        srcT_dram.rearrange("(o p) s -> p o s", p=128)
                        [:, :, sc * scw:(sc + 1) * scw])
                    kstage = stage.tile([128, c.DC, scw], F16, tag="kt_stage")
                    for mo in range(c.DC):
                        ps = pp.tile([128, scw], F32, tag="kv_ps")
                        for kc in range(c.DC):
                            mm(ps, wk_sb[:, kc, mo * 128:(mo + 1) * 128], xsb[:, kc, :],
                               start=(kc == 0), stop=(kc == c.DC - 1))
                        nc.scalar.activation(out=kstage[:, mo, :], in_=ps,
                                             func=AF.Identity,
                                             bias=bk_col[:, mo:mo + 1], scale=1.0)
                    nc.gpsimd.dma_start(
                        kt_dram.rearrange("m p s -> p m s")
                        [:, :, sc * scw:(sc + 1) * scw],
                        kstage[:])
                    vbig = stage.tile([128, nkk, c.H, c.HD], F16, tag="v_stage")
                    for kk in range(nkk):
                        for vo in range(nvo):
                            ps = pp.tile([128, voc], F32, tag="kv_ps")
                            for kc in range(c.DC):
                                mm(ps, xsb[:, kc, kk * 128:(kk + 1) * 128],
                                   wv_sb[:, kc, vo * voc:(vo + 1) * voc],
                                   start=(kc == 0), stop=(kc == c.DC - 1))
                            nc.vector.tensor_add(
                                out=vbig[:, kk, vo * hper:(vo + 1) * hper, :]
                                .rearrange("p h e -> p (h e)"),
                                in0=ps,
                                in1=bvb[:, vo * voc:(vo + 1) * voc])
                    for kk in range(nkk):
                        nc.gpsimd.dma_start(
                            vp_dram[sc * nkk + kk]
                            .rearrange("p (h e) -> p h e", e=c.HD + 1)[:, :, :c.HD],
                            vbig[:, kk])

        def attention(qT, kt_dram, vp_dram, mask_sb, ctxT):
            """Per-head pipeline: scores (PE) -> psum->PT fp16 (DVE, +mask) ->
            one whole-head exp (ACT) -> PV accumulate (PE) -> normalize.

            Heads pipeline against each other: while ACT runs head h's exp,
            the PE runs head h+1's scores and head h-1's PV matmuls."""
            G = min(4, c.KC)
            ng = c.KC // G
            hd = c.HD
            with tc.tile_pool(name="att_kv", bufs=2) as kvp, \
                 tc.tile_pool(name="att_pt", bufs=3) as ptp, \
                 tc.tile_pool(name="att_rd", bufs=2) as rdp, \
                 tc.tile_pool(name="att_ps_s", bufs=1, space="PSUM") as pps, \
                 tc.tile_pool(name="att_ps_pv", bufs=3, space="PSUM") as ppv:
                G2 = min(2, c.KC)
                ng2 = c.KC // G2
                for pair in range(c.DC):
                    eng = nc.gpsimd if pair % 2 == 0 else nc.sync
                    kt_sb = kvp.tile([128, c.SKV], F16, tag="kt_pair")
                    eng.dma_start(kt_sb[:], kt_dram[pair][:, :])
                    vp_sb = kvp.tile([128, c.KC, 2 * (hd + 1)], F16, tag="vp_pair")
                    eng.dma_start(
                        vp_sb[:],
                        vp_dram[:, :, pair * 2 * (hd + 1):(pair + 1) * 2 * (hd + 1)]
                        .rearrange("k p e -> p k e"))
                    # scores for BOTH heads, interleaved so the (0,0)/(64,0)
                    # row-group matmul pairs run concurrently in the array
                    pts = []
                    for hl in range(2):
                        pts.append(ptp.tile([128, c.KC, c.R], F16, tag="pt",
                                            name=f"pt{hl}"))
                    for g in range(ng2):
                        pss = []
                        for hl in range(2):
                            pss.append(pps.tile([128, G2, c.R], F32,
                                                tag=f"score_ps{hl}",
                                                name=f"score_ps{hl}"))
                        for kk in range(G2):
                            kc = g * G2 + kk
                            for hl in range(2):
                                h0 = hl * hd
                                mm(pss[hl][:, kk, :],
                                   kt_sb[h0:h0 + hd, kc * 128:(kc + 1) * 128],
                                   qT[h0:h0 + hd, pair, :])
                        kc0 = g * G2
                        for hl in range(2):
                            if mask_sb is not None:
                                # drain psum -> PT with the mask added (DVE)
                                nc.vector.tensor_add(
                                    out=pts[hl][:, kc0:kc0 + G2, :],
                                    in0=pss[hl][:],
                                    in1=mask_sb[:, kc0:kc0 + G2, :])
                            else:
                                # no mask: exp straight out of PSUM (ACT)
                                nc.scalar.activation(
                                    out=pts[hl][:, kc0:kc0 + G2, :],
                                    in_=pss[hl][:], func=AF.Exp, scale=scale)
                    rd = rdp.tile([128, c.R], F32, tag="rd")
                    ps_pvs = []
                    for hl in range(2):
                        h0 = hl * hd
                        pt = pts[hl]
                        if mask_sb is not None:
                            nc.scalar.activation(out=pt[:], in_=pt[:],
                                                 func=AF.Exp, scale=scale)
                        ps_pv = ppv.tile([128, c.R], F32, tag="pv_ps")
                        ps_pvs.append(ps_pv)
                        for kc in range(c.KC):
                            mm(ps_pv[0:hd + 1, :],
                               vp_sb[:, kc, hl * (hd + 1):(hl + 1) * (hd + 1)],
                               pt[:, kc, :],
                               start=(kc == 0), stop=(kc == c.KC - 1))
                        # stack both heads' denominators into one [128, R] tile
                        drow = rdp.tile([1, c.R], F32, tag="drow",
                                        name=f"drow{hl}")
                        nc.vector.tensor_copy(out=drow, in_=ps_pv[hd:hd + 1, :])
                        bcast_rows(rd[h0:h0 + hd, :], drow[0:1, :], c.R)
                    nc.vector.reciprocal(out=rd, in_=rd)
                    for hl in range(2):
                        h0 = hl * hd
                        nc.vector.tensor_mul(out=ctxT[h0:h0 + hd, pair, :],
                                             in0=ps_pvs[hl][0:hd, :],
                                             in1=rd[h0:h0 + hd, :])

        def attention_block(prefix, srcT_dram, queryT, wq, wk, wv, wo,
                            bq, bk, bvn, bo, kt_dram, vp_dram, mask_ap,
                            ln_g, ln_b, resid_inT, outT):
            with tc.tile_pool(name=f"{prefix}_act", bufs=1) as actp:
                qT = actp.tile([128, c.DC, c.R], F16, tag="qT")
                ctxT = actp.tile([128, c.DC, c.R], F16, tag="ctxT")
                with tc.tile_pool(name=f"{prefix}_w", bufs=1) as wpool:
                    with tc.tile_pool(name=f"{prefix}_ps", bufs=3, space="PSUM") as pp:
                        wq_sb = load_weight(wpool, W[wq], tag="wa")
                        for mo in range(c.DC):
                            ps = pp.tile([128, c.R], F32, tag="proj_ps")
                            for kc in range(c.DC):
                                mm(ps, wq_sb[:, kc, mo * 128:(mo + 1) * 128],
                                   queryT[:, kc, :],
                                   start=(kc == 0), stop=(kc == c.DC - 1))
                            nc.scalar.activation(out=qT[:, mo, :], in_=ps,
                                                 func=AF.Identity,
                                                 bias=cols[bq][:, mo:mo + 1], scale=1.0)
                    wk_sb = load_weight(wpool, W[wk], tag="wb")
                    wv_sb = load_weight(wpool, W[wv], tag="wa")
                    kv_proj(srcT_dram, wk_sb, wv_sb, cols[bk], bv_bc[bvn],
                            kt_dram, vp_dram)
                if mask_ap is not None:
                    with tc.tile_pool(name=f"{prefix}_mask", bufs=1) as mp:
                        mask_sb = mp.tile([128, c.KC, c.R], F16, tag="mask")
                        nc.sync.dma_start(
                            mask_sb[:], mask_ap.rearrange("(k p) q -> p k q", p=128))
                        attention(qT, kt_dram, vp_dram, mask_sb, ctxT)
                else:
                    attention(qT, kt_dram, vp_dram, None, ctxT)
                with tc.tile_pool(name=f"{prefix}_w2", bufs=2) as wpool, \
                     tc.tile_pool(name=f"{prefix}_st", bufs=3) as stage, \
                     tc.tile_pool(name=f"{prefix}_sm", bufs=2) as smallp, \
                     tc.tile_pool(name=f"{prefix}_bc", bufs=1) as bcp, \
                     tc.tile_pool(name=f"{prefix}_ps2", bufs=3, space="PSUM") as pp2:
                    wo_sb = load_weight(wpool, W[wo])
                    rT = actp.tile([128, c.DC, c.R], F16, tag="rT")
                    for mo in range(c.DC):
                        ps = pp2.tile([128, c.R], F32, tag="proj_ps")
                        for kc in range(c.DC):
                            mm(ps, wo_sb[:, kc, mo * 128:(mo + 1) * 128],
                               ctxT[:, kc, :], start=(kc == 0), stop=(kc == c.DC - 1))
                        t = stage.tile([128, c.R], F32, tag="o_t")
                        nc.scalar.activation(out=t, in_=ps, func=AF.Identity,
                                             bias=cols[bo][:, mo:mo + 1], scale=1.0)
                        nc.vector.tensor_add(out=rT[:, mo, :], in0=t, in1=resid_inT(mo))
                    layernorm({"small": smallp, "stage": stage, "bcast": bcp},
                              rT, outT, cols[ln_g], cols[ln_b])

        # ---- Phase A: self attention ----
        with tc.tile_pool(name="xq", bufs=1) as xqp:
            xTq_sb = xqp.tile([128, c.DC, c.R], F16, tag="xTq")
            nc.sync.dma_start(xTq_sb[:], xTq.rearrange("(o p) q -> p o q", p=128))
            attention_block(
                "sa", xT, xTq_sb, "wq_s", "wk_s", "wv_s", "wo_s",
                "bq_s", "bk_s", "bv_s", "bo_s", kt_s, vp_s, maskT,
                "ln1_g", "ln1_b", lambda mo: xTq_sb[:, mo, :], h1T)

        # ---- Phase B: cross attention ----
        attention_block(
            "ca", encT, h1T, "wq_c", "wk_c", "wv_c", "wo_c",
            "bq_c", "bk_c", "bv_c", "bo_c", kt_c, vp_c, maskcT,
            "ln2_g", "ln2_b", lambda mo: h1T[:, mo, :], h2T)

        # ---- Phase C: FFN ----
        with tc.tile_pool(name="ffg", bufs=1) as gp, \
             tc.tile_pool(name="ffw1", bufs=2) as wpool1, \
             tc.tile_pool(name="ffw2", bufs=3) as wpool2, \
             tc.tile_pool(name="ffst", bufs=3) as stage, \
             tc.tile_pool(name="ffsm", bufs=2) as smallp, \
             tc.tile_pool(name="ffbc", bufs=1) as bcp:
            gT = gp.tile([128, c.FC, c.R], F16, tag="gT")
            with tc.tile_pool(name="ffps1", bufs=3, space="PSUM") as pp:
                fw = W["ff1_w"].rearrange("(o p) n -> p o n", p=128)
                fblk = min(512, c.DFF)
                for fb in range(c.DFF // fblk):
                    w_sb = wpool1.tile([128, c.DC, fblk], F16, tag="ff1w")
                    dma_engs[fb % 3].dma_start(
                        w_sb[:], fw[:, :, fb * fblk:(fb + 1) * fblk])
                    for fl in range(fblk // 128):
                        fo = fb * (fblk // 128) + fl
                        ps = pp.tile([128, c.R], F32, tag="proj_ps")
                        for kc in range(c.DC):
                            mm(ps, w_sb[:, kc, fl * 128:(fl + 1) * 128],
                               h2T[:, kc, :], start=(kc == 0), stop=(kc == c.DC - 1))
                        nc.scalar.activation(out=gT[:, fo, :], in_=ps, func=AF.Relu,
                                             bias=cols["ff1_b"][:, fo:fo + 1], scale=1.0)
            rT = gp.tile([128, c.DC, c.R], F16, tag="ff_rT")
            with tc.tile_pool(name="ffps2", bufs=1, space="PSUM") as pp2:
                f2 = W["ff2_w"].rearrange("(o p) n -> p o n", p=128)
                ps_list = [pp2.tile([128, c.R], F32, tag=f"ff2_ps{mo}",
                                    name=f"ff2_ps{mo}")
                           for mo in range(c.DC)]
                fcg = 4  # fc chunks per weight-load DMA
                for fb in range(c.FC // fcg):
                    w_sb = wpool2.tile([128, fcg, c.D], F16, tag="ff2w")
                    dma_engs[fb % 3].dma_start(w_sb[:], f2[:, fb * fcg:(fb + 1) * fcg, :])
                    for fl in range(fcg):
                        fc = fb * fcg + fl
                        for mo in range(c.DC):
                            mm(ps_list[mo], w_sb[:, fl, mo * 128:(mo + 1) * 128],
                               gT[:, fc, :], start=(fc == 0), stop=(fc == c.FC - 1))
                for mo in range(c.DC):
                    t = stage.tile([128, c.R], F32, tag="ff_t")
                    nc.scalar.activation(out=t, in_=ps_list[mo], func=AF.Identity,
                                         bias=cols["ff2_b"][:, mo:mo + 1], scale=1.0)
                    nc.vector.tensor_add(out=rT[:, mo, :], in0=t, in1=h2T[:, mo, :])
            layernorm({"small": smallp, "stage": stage, "bcast": bcp},
                      rT, h3T, cols["ln3_g"], cols["ln3_b"])

        # ---- Phase D: transpose back to row-major and store ----
        with tc.tile_pool(name="outp", bufs=2) as op, \
             tc.tile_pool(name="outps", bufs=2, space="PSUM") as opp:
            for qc in range(c.R // 128):
                osb = op.tile([128, c.D], F32, tag="out_sb")
                for mo in range(c.DC):
                    pst = opp.tile([128, 128], F32, tag="t_ps")
                    nc.tensor.transpose(pst, h3T[:, mo, qc * 128:(qc + 1) * 128],
                                        identity)
                    nc.scalar.copy(out=osb[:, mo * 128:(mo + 1) * 128], in_=pst)
                nc.sync.dma_start(out[qc * 128:(qc + 1) * 128, :], osb[:])

    if use_bacc:
        nc.compile()
    return nc


# ---------------------------------------------------------------------------
# host-side driver
# ---------------------------------------------------------------------------

_CACHE: dict = {}
_LOCK = threading.Lock()


def _get_program(cfg: Cfg) -> bass.Bass:
    with _LOCK:
        if cfg not in _CACHE:
            _CACHE[cfg] = build_program(cfg)
        return _CACHE[cfg]


def make_in_maps(cfg: Cfg, inp: dict):
    c = cfg
    x = np.asarray(inp["x"]).astype(np.float32, copy=False)
    enc = np.asarray(inp["encoder_out"]).astype(np.float32, copy=False)
    tgt = np.asarray(inp["tgt_mask"])[0, 0]
    src = np.asarray(inp["src_mask"])[0, 0]
    R = c.R
    cores_per_batch = c.n_cores // c.B
    xT = [np.ascontiguousarray(x[bb].T.astype(np.float16)) for bb in range(c.B)]
    encT = [np.ascontiguousarray(enc[bb].T.astype(np.float16)) for bb in range(c.B)]
    wnames = ["wq_s", "wk_s", "wv_s", "wo_s", "wq_c", "wk_c", "wv_c", "wo_c",
              "ff1_w", "ff2_w"]
    bnames = ["bq_s", "bk_s", "bv_s", "bo_s", "bq_c", "bk_c", "bv_c", "bo_c",
              "ff1_b", "ff2_b", "ln1_g", "ln1_b", "ln2_g", "ln2_b",
              "ln3_g", "ln3_b"]
    shared = {}
    for n in wnames:
        shared[n] = np.ascontiguousarray(np.asarray(inp[n]).astype(np.float16))
    for n in bnames:
        shared[n] = np.ascontiguousarray(
            np.asarray(inp[n]).astype(np.float32, copy=False))
    shared["ones_in"] = np.ones((128, c.H), np.float16)
    in_maps = []
    for cid in range(c.n_cores):
        bb = cid // cores_per_batch
        j = cid % cores_per_batch
        q0 = j * R
        m = dict(shared)
        m["xT"] = xT[bb]
        m["encT"] = encT[bb]
        m["xTq"] = np.ascontiguousarray(xT[bb][:, q0:q0 + R])
        if c.use_self_mask:
            biasT = np.where(tgt[q0:q0 + R, :].T == 0,
                             np.float16(MASK_NEG), np.float16(0.0))
            m["maskT"] = np.ascontiguousarray(biasT)
        if c.use_cross_mask:
            biasT = np.where(src[q0:q0 + R, :].T == 0,
                             np.float16(MASK_NEG), np.float16(0.0))
            m["maskcT"] = np.ascontiguousarray(biasT)
        in_maps.append(m)
    return in_maps


def assemble_out(cfg: Cfg, results) -> np.ndarray:
    c = cfg
    cores_per_batch = c.n_cores // c.B
    out = np.empty((c.B, c.SQ, c.D), np.float32)
    for cid in range(c.n_cores):
        bb = cid // cores_per_batch
        j = cid % cores_per_batch
        out[bb, j * c.R:(j + 1) * c.R, :] = results[cid]["out"]
    return out


def cfg_from_inputs(inp: dict) -> Cfg:
    x = np.asarray(inp["x"])
    enc = np.asarray(inp["encoder_out"])
    tgt = np.asarray(inp["tgt_mask"])[0, 0]
    src = np.asarray(inp["src_mask"])[0, 0]
    return Cfg(B=x.shape[0], SQ=x.shape[1], SKV=enc.shape[1], D=x.shape[2],
               H=16, DFF=np.asarray(inp["ff1_w"]).shape[1], n_cores=8,
               use_self_mask=not bool((tgt != 0).all()),
               use_cross_mask=not bool((src != 0).all()))


def kernel(**inputs) -> np.ndarray:
    inp = {k: np.asarray(v) for k, v in inputs.items()}
    cfg = cfg_from_inputs(inp)
    nc = _get_program(cfg)
    in_maps = make_in_maps(cfg, inp)
    res = run_bass_kernel_spmd(nc, in_maps, core_ids=list(range(cfg.n_cores)))
    return assemble_out(cfg, res.results)

